# Initial kernel scaffold
#
"""FAENet GNN message-passing kernel for 8x Trainium2 NeuronCores (Bass/Tile).

Strategy (per sharding hint, adapted):
  - Nodes sharded contiguously across 8 cores (NSH rows each). Edges bucketed
    by OWNER CORE OF DST and, within a core, by 128-row node tile of dst, so
    the segment-sum (scatter-add) is purely local: per edge-tile a 0/1
    indicator matrix S (built on-device by iota-compare) and a TensorE matmul
    S^T @ m accumulate messages into PSUM per node tile. No [N,H] all-reduce.
  - h kept in a replicated DRAM "fat table" [NPAD, 256] bf16 = [h | A] where
    A = h @ Wgs^T + b_geom (the src-side projection of the edge MLP). Per
    layer each core computes its shard of h/A, and an AllGather rebuilds the
    table. h[src]/A[src] are fetched by indirect DMA row gathers; A and the
    local dst-side projection B = h @ Wgd^T are CCE-added during the gather
    directly onto the E-part staging, so W_pre = E + A[src] + B[dst] costs no
    vector-engine adds.
  - Edge MLP: e^T = silu(We_ext @ edge_in^T) recomputed per layer (K=7
    matmul); E_pre = e @ Wge^T per 128-edge tile (lhsT = e^T slice);
    W = silu(W_pre); m = h[src] * W; scatter via S-matmul into PSUM.
  - Node phase runs transposed [feat, node]: GraphNorm stats by free-axis
    reduction + tiny [128,2] AllReduce; gn+swish fused into one ACT op
    (silu(x*scale+bias)); node MLPs are weight-stationary matmuls.
  - Output block (energy head) computed on device per layer into per-node
    contributions; the final [G]-sized segment-sum by graph id and the last
    [G,5]@[5,1] linear run on host.

kernel(**inputs) takes the FULL inputs (as produced by setup_inputs) and
returns (energy [G,1], h [N,H]) as float32 numpy arrays.
"""

import math
from dataclasses import dataclass

import numpy as np
import ml_dtypes

import concourse.bass as bass
import concourse.mybir as mybir
import concourse.tile as tile
from concourse.bass_utils import run_bass_kernel_spmd
from concourse.masks import make_identity

BF16 = ml_dtypes.bfloat16
P = 128
H = 128
NF = 128
EPS = 1e-5


@dataclass(frozen=True)
class Cfg:
    NCORES: int = 8
    N: int = 50000          # real nodes
    G: int = 64             # graphs
    L: int = 4              # interaction layers
    NT: int = 49            # node tiles per core
    TPN: int = 18           # edge tiles per node tile (capacity)
    ET: int = 896           # edge tiles per core (>= NT*TPN, mult of BLKT*? and GATH_T)
    BLKT: int = 4           # edge tiles per compute block
    GATH_T: int = 32        # edge tiles per gather instruction
    CHUNK: int = 512        # free-dim chunk for node-phase ops

    @property
    def NSH(self):
        return self.NT * P

    @property
    def NPAD(self):
        return self.NCORES * self.NSH

    @property
    def ES(self):
        return self.ET * P

    @property
    def NBLK(self):
        return self.ET // self.BLKT

    @property
    def NG(self):
        return self.ET // self.GATH_T

    def chunks(self):
        out = []
        o = 0
        while o < self.NSH:
            c = min(self.CHUNK, self.NSH - o)
            out.append((o, c))
            o += c
        return out


FULL = Cfg()

F32 = mybir.dt.float32
BF = mybir.dt.bfloat16
I32 = mybir.dt.int32


def input_specs(cfg: Cfg):
    """name -> (shape, np dtype) of per-core device inputs."""
    L = cfg.L
    return {
        "einT": ([7, cfg.ES], np.float32),
        "node_inT": ([5, cfg.NSH], np.float32),
        "src_idx": ([P, cfg.ET], np.int32),
        "dst_idx": ([P, cfg.ET], np.int32),
        "dstcol": ([P, cfg.ET], np.float32),
        "we_ext": ([7, P], np.float32),
        "wh_ext": ([5, P], np.float32),
        "wge_t": ([L * P, H], BF16),   # w_geom[:, :, 0:NF] transposed per layer
        "wgs_t": ([L * P, H], BF16),   # src part
        "wgd_t": ([L * P, H], BF16),   # dst part
        "wlinh_t": ([L * P, H], BF16),
        "wmlp_t": ([L * P, H], BF16),
        "wout_pack": ([P, 65], BF16),  # [w_lin1^T | w_wlin^T]
        "wlin2_t": ([64, 1], BF16),
        "bgeom_c": ([P, L], np.float32),
        "blinh_c": ([P, L], np.float32),
        "bmlp_c": ([P, L], np.float32),
        "gnw_c": ([P, L], np.float32),
        "gnb_c": ([P, L], np.float32),
        "gnms_c": ([P, L], np.float32),
        "blin1_c": ([64, 1], np.float32),
        "bscal": ([1, 2], np.float32),  # [b_wlin, b_lin2]
    }


def output_specs(cfg: Cfg):
    return {
        "out_h": ([cfg.NSH, H], np.float32),
        "out_contrib": ([cfg.L + 1, cfg.NSH], np.float32),
    }


def emit(tc, io, cfg: Cfg):
    """Emit the whole program. io: dict name -> bass.AP (DRAM)."""
    nc = tc.nc
    L, NT, TPN, ET, BLKT, GT = cfg.L, cfg.NT, cfg.TPN, cfg.ET, cfg.BLKT, cfg.GATH_T
    NSH, NPAD = cfg.NSH, cfg.NPAD
    BPG = GT // BLKT  # blocks per gather group
    chunks = cfg.chunks()
    rg = [list(range(cfg.NCORES))]

    # per node tile: first/last edge tile index
    grp_start = [t * TPN for t in range(NT)]
    grp_end = [(t + 1) * TPN - 1 for t in range(NT)]
    grp_end[NT - 1] = ET - 1

    def tile_nt(j):
        return min(j // TPN, NT - 1)

    with (
        tc.tile_pool(name="const", bufs=1) as const,
        tc.tile_pool(name="big", bufs=1) as big,
        tc.tile_pool(name="stage", bufs=2) as stage,
        tc.tile_pool(name="work", bufs=3) as work,
        tc.tile_pool(name="small", bufs=4) as small,
        tc.tile_pool(name="ps", bufs=3, space="PSUM") as ps_big,
        tc.tile_pool(name="ps_msg", bufs=2, space="PSUM") as ps_msg,
        tc.tile_pool(name="ps_tr", bufs=2, space="PSUM") as ps_tr,
        tc.tile_pool(name="dram", bufs=1, space="DRAM") as dram,
    ):
        # ---------------- constants ----------------
        ident_f = const.tile([P, P], F32)
        make_identity(nc, ident_f[:])
        ident_b = const.tile([P, P], BF)
        nc.vector.tensor_copy(ident_b[:], ident_f[:])
        iota_mat = const.tile([P, P], F32)
        nc.gpsimd.iota(iota_mat[:], pattern=[[1, P]], base=0, channel_multiplier=0,
                       allow_small_or_imprecise_dtypes=True)

        def load_const(name):
            shp, _ = input_specs(cfg)[name]
            t = const.tile(shp, io[name].dtype)
            nc.sync.dma_start(t[:], io[name][:])
            return t

        we_ext = load_const("we_ext")
        wh_ext = load_const("wh_ext")
        wge_t = load_const("wge_t")
        wgs_t = load_const("wgs_t")
        wgd_t = load_const("wgd_t")
        wlinh_t = load_const("wlinh_t")
        wmlp_t = load_const("wmlp_t")
        wout_pack = load_const("wout_pack")
        wlin2_t = load_const("wlin2_t")
        bgeom_c = load_const("bgeom_c")
        blinh_c = load_const("blinh_c")
        bmlp_c = load_const("bmlp_c")
        gnw_c = load_const("gnw_c")
        gnb_c = load_const("gnb_c")
        gnms_c = load_const("gnms_c")
        blin1_c = load_const("blin1_c")
        bscal = load_const("bscal")
        src_idx = load_const("src_idx")
        dst_idx = load_const("dst_idx")
        dstcol = load_const("dstcol")
        node_inT = load_const("node_inT")

        # ---------------- persistent big tiles ----------------
        hT = big.tile([P, NSH], BF)          # current node features, transposed
        msgT = big.tile([P, NSH], F32)       # aggregated messages, transposed
        AT = big.tile([P, NSH], BF)
        BT = big.tile([P, NSH], BF)
        sq_scr = big.tile([P, NSH], BF)      # scratch for square/stat pass

        # DRAM intermediates
        fat_shard = dram.tile([NSH, 2 * H], BF)
        fat_table = dram.tile([NPAD, 2 * H], BF)
        b_local = dram.tile([NSH, H], BF)
        stats_in = dram.tile([P, 2], F32)
        stats_out = dram.tile([P, 2], F32)

        Silu = mybir.ActivationFunctionType.Silu
        Ident = mybir.ActivationFunctionType.Identity
        Sqrt = mybir.ActivationFunctionType.Sqrt
        Square = mybir.ActivationFunctionType.Square

        # ---------------- embedding: hT = silu(wh_ext @ node_inT) ----------------
        for (o, c) in chunks:
            pse = ps_big.tile([P, cfg.CHUNK], F32, tag="psb")
            nc.tensor.matmul(pse[:, :c], lhsT=wh_ext[:], rhs=node_inT[:, o:o + c],
                             start=True, stop=True)
            nc.scalar.activation(hT[:, o:o + c], pse[:, :c], Silu)

        # ---------------- layers ----------------
        for l in range(L + 1):
            # ---- output block: contrib_l from hT ----
            tmpT = big.tile([64, NSH], BF, tag="tmpT")
            alphaT = big.tile([1, NSH], F32, tag="alphaT")
            t2T = big.tile([1, NSH], F32, tag="t2T")
            for (o, c) in chunks:
                pso = ps_big.tile([65, cfg.CHUNK], F32, tag="psb")
                nc.tensor.matmul(pso[:, :c], lhsT=wout_pack[:], rhs=hT[:, o:o + c],
                                 start=True, stop=True)
                nc.scalar.activation(tmpT[:, o:o + c], pso[:64, :c], Silu,
                                     bias=blin1_c[:, :])
                nc.scalar.activation(alphaT[:, o:o + c], pso[64:65, :c], Ident,
                                     bias=bscal[:, 0:1])
            for (o, c) in chunks:
                ps2 = ps_big.tile([1, cfg.CHUNK], F32, tag="ps1")
                nc.tensor.matmul(ps2[:, :c], lhsT=wlin2_t[:], rhs=tmpT[:, o:o + c],
                                 start=True, stop=True)
                nc.scalar.activation(t2T[:, o:o + c], ps2[:, :c], Ident,
                                     bias=bscal[:, 1:2])
            contribT = big.tile([1, NSH], F32, tag="contribT")
            nc.vector.tensor_mul(contribT[:], t2T[:], alphaT[:])
            nc.sync.dma_start(io["out_contrib"][l:l + 1, :], contribT[:])

            if l == L:
                # final h rows out (f32)
                for t in range(NT):
                    pst = ps_tr.tile([P, P], F32, tag="pst")
                    nc.tensor.transpose(pst[:], hT[:, t * P:(t + 1) * P], ident_b[:])
                    rowf = work.tile([P, P], F32, tag="rowf")
                    nc.vector.tensor_copy(rowf[:], pst[:])
                    nc.sync.dma_start(io["out_h"][t * P:(t + 1) * P, :], rowf[:])
                break

            lp = slice(l * P, (l + 1) * P)

            # ---- A^T, B^T ----
            for (o, c) in chunks:
                psa = ps_big.tile([P, cfg.CHUNK], F32, tag="psb")
                nc.tensor.matmul(psa[:, :c], lhsT=wgs_t[lp, :], rhs=hT[:, o:o + c],
                                 start=True, stop=True)
                nc.scalar.activation(AT[:, o:o + c], psa[:, :c], Ident,
                                     bias=bgeom_c[:, l:l + 1])
            for (o, c) in chunks:
                psb = ps_big.tile([P, cfg.CHUNK], F32, tag="psb")
                nc.tensor.matmul(psb[:, :c], lhsT=wgd_t[lp, :], rhs=hT[:, o:o + c],
                                 start=True, stop=True)
                nc.scalar.activation(BT[:, o:o + c], psb[:, :c], Ident)

            # ---- transpose h/A/B to rows; fill fat_shard + b_local ----
            for t in range(NT):
                sl = slice(t * P, (t + 1) * P)
                for src_t, dst_dram, col0 in (
                    (hT, fat_shard, 0),
                    (AT, fat_shard, H),
                    (BT, b_local, None),
                ):
                    pst = ps_tr.tile([P, P], F32, tag="pst")
                    nc.tensor.transpose(pst[:], src_t[:, sl], ident_b[:])
                    rowb = work.tile([P, P], BF, tag="rowb")
                    nc.vector.tensor_copy(rowb[:], pst[:])
                    if col0 is None:
                        nc.sync.dma_start(dst_dram[sl, :], rowb[:])
                    else:
                        nc.sync.dma_start(dst_dram[sl, col0:col0 + H], rowb[:])

            # ---- AllGather fat table ----
            nc.gpsimd.collective_compute(
                "AllGather", mybir.AluOpType.bypass, replica_groups=rg,
                ins=[fat_shard.opt()], outs=[fat_table.opt()],
            )

            # ---- edge loop ----
            for g in range(cfg.NG):
                gsl = slice(g * GT, (g + 1) * GT)
                h_stage = stage.tile([P, GT * H], BF, tag="h_stage")
                nc.gpsimd.indirect_dma_start(
                    out=h_stage[:], out_offset=None, in_=fat_table[:],
                    in_offset=bass.IndirectOffsetOnAxis(ap=src_idx[:, gsl], axis=0),
                )
                staging = stage.tile([P, GT * H], BF, tag="staging")
                # E_pre per block -> staging
                for bb in range(BPG):
                    b = g * BPG + bb
                    esl = slice(b * BLKT * P, (b + 1) * BLKT * P)
                    ein_sb = work.tile([7, BLKT * P], F32, tag="ein")
                    nc.sync.dma_start(ein_sb[:], io["einT"][:, esl])
                    pse = ps_big.tile([P, BLKT * P], F32, tag="pse")
                    nc.tensor.matmul(pse[:], lhsT=we_ext[:], rhs=ein_sb[:],
                                     start=True, stop=True)
                    eT_sb = work.tile([P, BLKT * P], BF, tag="eT")
                    nc.scalar.activation(eT_sb[:], pse[:], Silu)
                    psw = ps_big.tile([P, BLKT * P], F32, tag="psw")
                    for k in range(BLKT):
                        ks = slice(k * P, (k + 1) * P)
                        nc.tensor.matmul(psw[:, ks], lhsT=eT_sb[:, ks],
                                         rhs=wge_t[lp, :], start=True, stop=True)
                    nc.vector.tensor_copy(
                        staging[:, bb * BLKT * P:(bb + 1) * BLKT * P], psw[:])
                # B[dst] and A[src] cce-added onto staging
                nc.gpsimd.indirect_dma_start(
                    out=staging[:], out_offset=None, in_=b_local[:],
                    in_offset=bass.IndirectOffsetOnAxis(ap=dst_idx[:, gsl], axis=0),
                    compute_op=mybir.AluOpType.add,
                )
                nc.gpsimd.indirect_dma_start(
                    out=staging[:], out_offset=None, in_=fat_table[:],
                    in_offset=bass.IndirectOffsetOnAxis(ap=src_idx[:, gsl], axis=0),
                    element_offset=H,
                    compute_op=mybir.AluOpType.add,
                )
                # silu + m-mul + scatter
                for bb in range(BPG):
                    b = g * BPG + bb
                    bsl = slice(bb * BLKT * P, (bb + 1) * BLKT * P)
                    w_sb = work.tile([P, BLKT * P], BF, tag="w_sb")
                    nc.scalar.activation(w_sb[:], staging[:, bsl], Silu)
                    m_sb = work.tile([P, BLKT * P], BF, tag="m_sb")
                    nc.vector.tensor_mul(m_sb[:], w_sb[:], h_stage[:, bsl])
                    for k in range(BLKT):
                        j = b * BLKT + k
                        t = tile_nt(j)
                        s_sb = work.tile([P, P], BF, tag="s_sb")
                        nc.vector.tensor_tensor(
                            out=s_sb[:],
                            in0=dstcol[:, j:j + 1].to_broadcast([P, P]),
                            in1=iota_mat[:], op=mybir.AluOpType.is_equal)
                        pm = ps_msg.tile([P, P], F32, tag=f"pm")
                        if j == grp_start[t]:
                            pm_cur = pm  # new accumulation group
                        nc.tensor.matmul(pm_cur[:], lhsT=s_sb[:],
                                         rhs=m_sb[:, k * P:(k + 1) * P],
                                         start=(j == grp_start[t]),
                                         stop=(j == grp_end[t]))
                        if j == grp_end[t]:
                            mrow = work.tile([P, P], F32, tag="mrow")
                            nc.scalar.copy(mrow[:], pm_cur[:])
                            pst = ps_tr.tile([P, P], F32, tag="pst")
                            nc.tensor.transpose(pst[:], mrow[:], ident_f[:])
                            nc.vector.tensor_copy(msgT[:, t * P:(t + 1) * P], pst[:])

            # ---- GraphNorm stats + AllReduce ----
            st = small.tile([P, 2], F32, tag="st")
            nc.vector.tensor_reduce(st[:, 0:1], msgT[:], axis=mybir.AxisListType.X,
                                    op=mybir.AluOpType.add)
            nc.scalar.activation(sq_scr[:], msgT[:], Square, accum_out=st[:, 1:2])
            nc.sync.dma_start(stats_in[:], st[:])
            nc.gpsimd.collective_compute(
                "AllReduce", mybir.AluOpType.add, replica_groups=rg,
                ins=[stats_in.opt()], outs=[stats_out.opt()],
            )
            stg = small.tile([P, 2], F32, tag="stg")
            nc.sync.dma_start(stg[:], stats_out[:])

            # scalars: alpha_c, beta_c  [P,1]
            inv_n = 1.0 / cfg.N
            m1 = small.tile([P, 1], F32, tag="m1")
            nc.vector.tensor_scalar_mul(m1[:], stg[:, 0:1], inv_n)
            t1 = small.tile([P, 1], F32, tag="t1")
            nc.vector.tensor_mul(t1[:], gnms_c[:, l:l + 1], m1[:])
            e2 = small.tile([P, 1], F32, tag="e2")
            nc.vector.tensor_scalar_mul(e2[:], stg[:, 1:2], inv_n)
            q = small.tile([P, 1], F32, tag="q")
            nc.vector.tensor_mul(q[:], m1[:], t1[:])
            p2 = small.tile([P, 1], F32, tag="p2")
            nc.vector.tensor_mul(p2[:], t1[:], t1[:])
            var = small.tile([P, 1], F32, tag="var")
            nc.vector.tensor_scalar(out=var[:], in0=q[:], scalar1=-2.0, scalar2=None,
                                    op0=mybir.AluOpType.mult)
            nc.vector.tensor_add(var[:], var[:], e2[:])
            nc.vector.tensor_add(var[:], var[:], p2[:])
            sd = small.tile([P, 1], F32, tag="sd")
            nc.scalar.activation(sd[:], var[:], Sqrt, bias=EPS)
            rinv = small.tile([P, 1], F32, tag="rinv")
            nc.vector.reciprocal(rinv[:], sd[:])
            alpha_c = small.tile([P, 1], F32, tag="alpha_c")
            nc.vector.tensor_mul(alpha_c[:], gnw_c[:, l:l + 1], rinv[:])
            bb_ = small.tile([P, 1], F32, tag="bb_")
            nc.vector.tensor_mul(bb_[:], t1[:], alpha_c[:])
            beta_c = small.tile([P, 1], F32, tag="beta_c")
            nc.vector.tensor_tensor(out=beta_c[:], in0=gnb_c[:, l:l + 1], in1=bb_[:],
                                    op=mybir.AluOpType.subtract)

            # ---- gn + swish + node MLPs ----
            x1 = big.tile([P, NSH], BF, tag="x1")
            for (o, c) in chunks:
                nc.scalar.activation(x1[:, o:o + c], msgT[:, o:o + c], Silu,
                                     bias=beta_c[:], scale=alpha_c[:])
            x2 = big.tile([P, NSH], BF, tag="x2")
            for (o, c) in chunks:
                ps1 = ps_big.tile([P, cfg.CHUNK], F32, tag="psb")
                nc.tensor.matmul(ps1[:, :c], lhsT=wlinh_t[lp, :], rhs=x1[:, o:o + c],
                                 start=True, stop=True)
                nc.scalar.activation(x2[:, o:o + c], ps1[:, :c], Silu,
                                     bias=blinh_c[:, l:l + 1])
            for (o, c) in chunks:
                ps2_ = ps_big.tile([P, cfg.CHUNK], F32, tag="psb")
                nc.tensor.matmul(ps2_[:, :c], lhsT=wmlp_t[lp, :], rhs=x2[:, o:o + c],
                                 start=True, stop=True)
                nc.scalar.activation(hT[:, o:o + c], ps2_[:, :c], Silu,
                                     bias=bmlp_c[:, l:l + 1])


# ====================== host side ======================

def _prep_weights(cfg: Cfg, w):
    L = cfg.L
    out = {}
    we = np.zeros((7, P), np.float32)
    we[0:3, 0:64] = w["w_e1"].T          # [64,3] -> [3,64]
    we[3:6, 64:128] = w["w_e12"].T       # [64,3] -> [3,64]
    we[6, 0:64] = w["b_e1"]
    we[6, 64:128] = w["b_e12"]
    out["we_ext"] = we
    wh = np.zeros((5, P), np.float32)
    wh[0:3, 0:64] = w["w_h1"].T
    wh[3:4, 64:128] = w["w_h12"].T
    wh[4, 0:64] = w["b_h1"]
    wh[4, 64:128] = w["b_h12"]
    out["wh_ext"] = wh

    def packT(mat):  # [L, out, in] -> [L*in, out] stacked transposes
        return np.concatenate([mat[i].T for i in range(L)], axis=0).astype(BF16)

    out["wge_t"] = packT(w["w_geom"][:, :, 0:NF])
    out["wgs_t"] = packT(w["w_geom"][:, :, NF:NF + H])
    out["wgd_t"] = packT(w["w_geom"][:, :, NF + H:NF + 2 * H])
    out["wlinh_t"] = packT(w["w_linh"])
    out["wmlp_t"] = packT(w["w_mlp"])
    wo = np.zeros((P, 65), np.float32)
    wo[:, 0:64] = w["w_lin1"].T
    wo[:, 64] = w["w_wlin"][0]
    out["wout_pack"] = wo.astype(BF16)
    out["wlin2_t"] = w["w_lin2"].T.astype(BF16)
    out["bgeom_c"] = w["b_geom"].T.astype(np.float32).copy()    # [P, L]
    out["blinh_c"] = w["b_linh"].T.astype(np.float32).copy()
    out["bmlp_c"] = w["b_mlp"].T.astype(np.float32).copy()
    out["gnw_c"] = w["gn_w"].T.astype(np.float32).copy()
    out["gnb_c"] = w["gn_b"].T.astype(np.float32).copy()
    out["gnms_c"] = w["gn_ms"].T.astype(np.float32).copy()
    out["blin1_c"] = w["b_lin1"][:, None].astype(np.float32)
    out["bscal"] = np.array([[w["b_wlin"][0], w["b_lin2"][0]]], np.float32)
    return out


def preprocess(cfg: Cfg, inputs):
    """Full inputs -> list of per-core in_maps + host-side leftovers."""
    pos = np.asarray(inputs["pos"], np.float32)
    forces = np.asarray(inputs["forces"], np.float32)
    beam = np.asarray(inputs["beam_col"], np.float32)
    ei = np.asarray(inputs["edge_index"]).astype(np.int64)
    batch = np.asarray(inputs["batch"]).astype(np.int64)
    src, dst = ei[0], ei[1]
    E = src.shape[0]
    NSH, ET, TPN, NT = cfg.NSH, cfg.ET, cfg.TPN, cfg.NT

    rel = pos[src] - pos[dst]
    ew = np.sqrt((rel * rel).sum(1))
    ein_all = np.concatenate(
        [rel, beam, ew[:, None], np.ones((E, 1), np.float32)], axis=1
    ).astype(np.float32)  # [E, 7]

    fn = np.sqrt((forces * forces).sum(1))
    node_in = np.concatenate(
        [forces, fn[:, None], np.ones((len(fn), 1), np.float32)], axis=1)  # [N,5]
    node_in_pad = np.zeros((cfg.NPAD, 5), np.float32)
    node_in_pad[:cfg.N] = node_in

    wmap = _prep_weights(cfg, inputs)

    owner = dst // NSH
    in_maps = []
    for c in range(cfg.NCORES):
        sel = np.nonzero(owner == c)[0]
        ldst = dst[sel] - c * NSH
        t_of = ldst >> 7
        # slots: group g occupies [t*TPN*128, ...)
        einT = np.zeros((7, cfg.ES), np.float32)
        src_i = np.full((cfg.ES,), cfg.N, np.int32)     # dummy -> zero-ish row
        dst_i = np.zeros((cfg.ES,), np.int32)
        dcol = np.full((cfg.ES,), -1.0, np.float32)
        for t in range(NT):
            gsel = sel[t_of == t]
            cnt = len(gsel)
            cap = TPN * P
            if cnt > cap:
                raise RuntimeError(f"core {c} node-tile {t}: {cnt} > cap {cap}")
            base = t * TPN * P
            einT[:, base:base + cnt] = ein_all[gsel].T
            src_i[base:base + cnt] = src[gsel]
            dst_i[base:base + cnt] = dst[gsel] - c * NSH
            dcol[base:base + cnt] = (dst[gsel] - c * NSH) & 127
        im = {
            "einT": einT,
            "node_inT": node_in_pad[c * NSH:(c + 1) * NSH].T.copy(),
            "src_idx": src_i.reshape(cfg.ET, P).T.copy(),
            "dst_idx": dst_i.reshape(cfg.ET, P).T.copy(),
            "dstcol": dcol.reshape(cfg.ET, P).T.copy(),
        }
        im.update(wmap)
        in_maps.append(im)
    return in_maps, batch


def postprocess(cfg: Cfg, inputs, results, batch):
    h = np.concatenate([r["out_h"] for r in results], axis=0)[:cfg.N]
    contrib = np.concatenate([r["out_contrib"] for r in results], axis=1)[:, :cfg.N]
    w_skip = np.asarray(inputs["w_skip"], np.float32)
    b_skip = np.asarray(inputs["b_skip"], np.float32)
    energies = np.zeros((cfg.G, cfg.L + 1), np.float32)
    for ll in range(cfg.L + 1):
        np.add.at(energies[:, ll], batch, contrib[ll])
    energy = energies @ w_skip.T + b_skip
    return energy.astype(np.float32), h.astype(np.float32)


_CACHE = {}


def build_nc(cfg: Cfg):
    nc = bass.Bass()
    io = {}
    for name, (shp, dt) in input_specs(cfg).items():
        io[name] = nc.dram_tensor(
            name, shp, mybir.dt.from_np(np.dtype(dt)), kind="ExternalInput").ap()
    for name, (shp, dt) in output_specs(cfg).items():
        io[name] = nc.dram_tensor(
            name, shp, mybir.dt.from_np(np.dtype(dt)), kind="ExternalOutput").ap()
    with tile.TileContext(nc) as tc:
        emit(tc, io, cfg)
    return nc


def kernel(**inputs):
    cfg = FULL
    in_maps, batch = preprocess(cfg, inputs)
    if "nc" not in _CACHE:
        _CACHE["nc"] = build_nc(cfg)
    res = run_bass_kernel_spmd(
        _CACHE["nc"], in_maps, core_ids=list(range(cfg.NCORES)))
    return postprocess(cfg, inputs, res.results, batch)


# revision 26
# speedup vs baseline: 1.6648x; 1.6648x over previous
"""FAENet GNN message-passing kernel for 8x Trainium2 NeuronCores (Bass/Tile).

Strategy (per sharding hint, adapted):
  - Nodes sharded contiguously across 8 cores (NSH rows each). Edges bucketed
    by OWNER CORE OF DST and, within a core, by 128-row node tile of dst, so
    the segment-sum (scatter-add) is purely local: per edge-tile a 0/1
    indicator matrix S (built on-device by iota-compare) and a TensorE matmul
    S^T @ m accumulate messages into PSUM per node tile. No [N,H] all-reduce.
  - h kept in a replicated DRAM "fat table" [NPAD, 256] bf16 = [h | A] where
    A = h @ Wgs^T + b_geom (the src-side projection of the edge MLP). Per
    layer each core computes its shard of h/A, and an AllGather rebuilds the
    table. h[src]/A[src] are fetched by indirect DMA row gathers; A and the
    local dst-side projection B = h @ Wgd^T are CCE-added during the gather
    directly onto the E-part staging, so W_pre = E + A[src] + B[dst] costs no
    vector-engine adds.
  - Edge MLP: e^T = silu(We_ext @ edge_in^T) recomputed per layer (K=7
    matmul); E_pre = e @ Wge^T per 128-edge tile (lhsT = e^T slice);
    W = silu(W_pre); m = h[src] * W; scatter via S-matmul into PSUM.
  - Node phase runs transposed [feat, node]: GraphNorm stats by free-axis
    reduction + tiny [128,2] AllReduce; gn+swish fused into one ACT op
    (silu(x*scale+bias)); node MLPs are weight-stationary matmuls.
  - Output block (energy head) computed on device per layer into per-node
    contributions; the final [G]-sized segment-sum by graph id and the last
    [G,5]@[5,1] linear run on host.

kernel(**inputs) takes the FULL inputs (as produced by setup_inputs) and
returns (energy [G,1], h [N,H]) as float32 numpy arrays.
"""

import math
from dataclasses import dataclass

import numpy as np
import ml_dtypes

import concourse.bass as bass
import concourse.bacc as bacc
import concourse.mybir as mybir
import concourse.tile as tile
from concourse.bass_utils import run_bass_kernel_spmd
from concourse.masks import make_identity

BF16 = ml_dtypes.bfloat16
P = 128
H = 128
NF = 128
EPS = 1e-5


@dataclass(frozen=True)
class Cfg:
    NCORES: int = 8
    N: int = 50000          # real nodes
    G: int = 64             # graphs
    L: int = 4              # interaction layers
    NT: int = 49            # node tiles per core
    TPNH: int = 9           # edge tiles per (node tile, src-half) group
    HT: int = 448           # edge tiles per src-half region (>= NT*TPNH, mult of GATH_T)
    BLKT: int = 4           # edge tiles per compute block
    GATH_T: int = 8         # edge tiles per gather instruction
    CHUNK: int = 512        # free-dim chunk for node-phase ops

    @property
    def NSH(self):
        return self.NT * P

    @property
    def NPAD(self):
        return self.NCORES * self.NSH

    @property
    def NHALF(self):
        return self.NPAD // 2

    @property
    def ET(self):
        return 2 * self.HT

    @property
    def ES(self):
        return self.ET * P

    @property
    def NBLK(self):
        return self.ET // self.BLKT

    @property
    def NG(self):
        return self.ET // self.GATH_T

    def chunks(self):
        out = []
        o = 0
        while o < self.NSH:
            c = min(self.CHUNK, self.NSH - o)
            out.append((o, c))
            o += c
        return out


FULL = Cfg()

F32 = mybir.dt.float32
BF = mybir.dt.bfloat16
I32 = mybir.dt.int32


def input_specs(cfg: Cfg):
    """name -> (shape, np dtype) of per-core device inputs."""
    L = cfg.L
    return {
        "einT": ([7, cfg.ES], np.float32),
        "node_inT": ([5, cfg.NSH], np.float32),
        "src_idx16": ([P, cfg.ES // 16], np.int16),
        "dst_idx16": ([P, cfg.ES // 16], np.int16),
        "dstcol": ([P, cfg.ET], np.float32),
        "we_ext": ([7, P], np.float32),
        "wh_ext": ([5, P], np.float32),
        "wge_t": ([P, L * H], BF16),   # w_geom[:, :, 0:NF] transposed per layer
        "wgs_t": ([P, L * H], np.float32),   # src part
        "wgd_t": ([P, L * H], np.float32),   # dst part
        "wlinh_t": ([P, L * H], BF16),
        "wmlp_t": ([P, L * H], BF16),
        "wout_pack": ([P, 65], np.float32),  # [w_lin1^T | w_wlin^T]
        "wlin2_t": ([64, 1], BF16),
        "bgeom_c": ([P, L], np.float32),
        "blinh_c": ([P, L], np.float32),
        "bmlp_c": ([P, L], np.float32),
        "gnw_c": ([P, L], np.float32),
        "gnb_c": ([P, L], np.float32),
        "gnms_c": ([P, L], np.float32),
        "blin1_c": ([64, 1], np.float32),
        "bscal": ([1, 2], np.float32),  # [b_wlin, b_lin2]
    }


def output_specs(cfg: Cfg):
    return {
        "out_h": ([cfg.NSH, H], np.float32),
        "out_contrib": ([cfg.L + 1, cfg.NSH], np.float32),
    }


def emit(tc, io, cfg: Cfg):
    """Emit the whole program. io: dict name -> bass.AP (DRAM)."""
    nc = tc.nc
    L, NT, TPNH, ET, BLKT, GT = cfg.L, cfg.NT, cfg.TPNH, cfg.ET, cfg.BLKT, cfg.GATH_T
    HT, NSH, NPAD, NHALF = cfg.HT, cfg.NSH, cfg.NPAD, cfg.NHALF
    BPG = GT // BLKT  # blocks per gather group
    chunks = cfg.chunks()
    rg = [list(range(cfg.NCORES))]

    # tile -> (node tile, half, is_group_start, is_group_end)
    def tile_meta(j):
        half = j // HT
        r = j - half * HT
        t = min(r // TPNH, NT - 1)
        start = r == t * TPNH
        end = (r == (t + 1) * TPNH - 1) if t < NT - 1 else (r == HT - 1)
        return t, half, start, end

    with (
        tc.tile_pool(name="const", bufs=1) as const,
        tc.tile_pool(name="big", bufs=1) as big,
        tc.tile_pool(name="stage", bufs=2) as stage,
        tc.tile_pool(name="work", bufs=3) as work,
        tc.tile_pool(name="small", bufs=4) as small,
        tc.tile_pool(name="ps", bufs=2, space="PSUM") as ps_big,
        tc.tile_pool(name="ps_msg", bufs=2, space="PSUM") as ps_msg,
        tc.tile_pool(name="ps_tr", bufs=2, space="PSUM") as ps_tr,
        tc.tile_pool(name="dram", bufs=1, space="DRAM") as dram,
    ):
        # ---------------- constants ----------------
        ident_f = const.tile([P, P], F32)
        make_identity(nc, ident_f[:])
        ident_b = const.tile([P, P], BF)
        nc.vector.tensor_copy(ident_b[:], ident_f[:])
        iota_mat = const.tile([P, P], F32)
        nc.gpsimd.iota(iota_mat[:], pattern=[[1, P]], base=0, channel_multiplier=0,
                       allow_small_or_imprecise_dtypes=True)

        def load_const(name):
            shp, _ = input_specs(cfg)[name]
            t = const.tile(shp, io[name].dtype, tag=name)
            nc.sync.dma_start(t[:], io[name][:])
            return t

        we_ext = load_const("we_ext")
        wh_ext = load_const("wh_ext")
        wge_t = load_const("wge_t")
        wgs_t = load_const("wgs_t")
        wgd_t = load_const("wgd_t")
        wlinh_t = load_const("wlinh_t")
        wmlp_t = load_const("wmlp_t")
        wout_pack = load_const("wout_pack")
        wlin2_t = load_const("wlin2_t")
        bgeom_c = load_const("bgeom_c")
        blinh_c = load_const("blinh_c")
        bmlp_c = load_const("bmlp_c")
        gnw_c = load_const("gnw_c")
        gnb_c = load_const("gnb_c")
        gnms_c = load_const("gnms_c")
        blin1_c = load_const("blin1_c")
        bscal = load_const("bscal")
        dstcol = load_const("dstcol")

        # ---------------- persistent big tiles ----------------
        hT = big.tile([P, NSH], F32)         # current node features, transposed

        # DRAM intermediates (collective outputs must be Shared Internal)
        fat_shard = dram.tile([NSH, 2 * H], F32)
        b_local = dram.tile([NSH, H], F32)
        stats_in = dram.tile([P, 2], F32)
        fat_table = nc.dram_tensor("fat_table_sh", [NPAD, 2 * H], F32,
                                   kind="Internal", addr_space="Shared").ap()
        stats_out = nc.dram_tensor("stats_out_sh", [P, 2], F32,
                                   kind="Internal", addr_space="Shared").ap()

        Silu = mybir.ActivationFunctionType.Silu
        Ident = mybir.ActivationFunctionType.Identity
        Sqrt = mybir.ActivationFunctionType.Sqrt
        Square = mybir.ActivationFunctionType.Square

        # ---------------- embedding: hT = silu(wh_ext @ node_inT) ----------------
        for (o, c) in chunks:
            ni_sb = work.tile([5, cfg.CHUNK], F32, tag="ni")
            nc.sync.dma_start(ni_sb[:, :c], io["node_inT"][:, o:o + c])
            pse = ps_big.tile([P, cfg.CHUNK], F32, tag="psb")
            nc.tensor.matmul(pse[:, :c], lhsT=wh_ext[:], rhs=ni_sb[:, :c],
                             start=True, stop=True)
            nc.scalar.activation(hT[:, o:o + c], pse[:, :c], Silu)

        # ---------------- layers ----------------
        for l in range(L + 1):
            # ---- output block: contrib_l from hT (per-chunk, low SBUF) ----
            for (o, c) in chunks:
                pso = ps_big.tile([65, cfg.CHUNK], F32, tag="psb")
                nc.tensor.matmul(pso[:, :c], lhsT=wout_pack[:], rhs=hT[:, o:o + c],
                                 start=True, stop=True)
                tmp_sb = work.tile([64, cfg.CHUNK], BF, tag="tmp_sb")
                nc.scalar.activation(tmp_sb[:, :c], pso[:64, :c], Silu,
                                     bias=blin1_c[:, :])
                al_sb = big.tile([1, cfg.CHUNK], F32, tag="al_sb")
                nc.scalar.activation(al_sb[:, :c], pso[64:65, :c], Ident,
                                     bias=bscal[:, 0:1])
                ps2 = ps_big.tile([1, cfg.CHUNK], F32, tag="psb")
                nc.tensor.matmul(ps2[:, :c], lhsT=wlin2_t[:], rhs=tmp_sb[:, :c],
                                 start=True, stop=True)
                t2_sb = big.tile([1, cfg.CHUNK], F32, tag="t2_sb")
                nc.scalar.activation(t2_sb[:, :c], ps2[:, :c], Ident,
                                     bias=bscal[:, 1:2])
                co_sb = big.tile([1, cfg.CHUNK], F32, tag="co_sb")
                nc.vector.tensor_mul(co_sb[:, :c], t2_sb[:, :c], al_sb[:, :c])
                nc.sync.dma_start(io["out_contrib"][l:l + 1, o:o + c], co_sb[:, :c])

            if l == L:
                # final h rows out (f32)
                for t in range(NT):
                    pst = ps_tr.tile([P, P], F32, tag="pst")
                    nc.tensor.transpose(pst[:], hT[:, t * P:(t + 1) * P], ident_f[:])
                    rowf = work.tile([P, P], F32, tag="rowf")
                    nc.vector.tensor_copy(rowf[:], pst[:])
                    nc.sync.dma_start(io["out_h"][t * P:(t + 1) * P, :], rowf[:])
                break

            lp = slice(l * H, (l + 1) * H)

            # ---- A^T, B^T ---- (share slots with msgT/msg_rows: disjoint lifetime)
            AT = big.tile([P, NSH], F32, tag="ovl_msgT")
            BT = big.tile([P, NSH], F32, tag="ovl_rows")
            for (o, c) in chunks:
                psa = ps_big.tile([P, cfg.CHUNK], F32, tag="psb")
                nc.tensor.matmul(psa[:, :c], lhsT=wgs_t[:, lp], rhs=hT[:, o:o + c],
                                 start=True, stop=True)
                nc.scalar.activation(AT[:, o:o + c], psa[:, :c], Ident,
                                     bias=bgeom_c[:, l:l + 1])
            for (o, c) in chunks:
                psb = ps_big.tile([P, cfg.CHUNK], F32, tag="psb")
                nc.tensor.matmul(psb[:, :c], lhsT=wgd_t[:, lp], rhs=hT[:, o:o + c],
                                 start=True, stop=True)
                nc.scalar.activation(BT[:, o:o + c], psb[:, :c], Ident)

            # ---- transpose h/A/B to rows; fill fat_shard + b_local ----
            for t in range(NT):
                sl = slice(t * P, (t + 1) * P)
                for src_t, dst_dram, col0 in (
                    (hT, fat_shard, 0),
                    (AT, fat_shard, H),
                    (BT, b_local, None),
                ):
                    pst = ps_tr.tile([P, P], F32, tag="pst")
                    nc.tensor.transpose(pst[:], src_t[:, sl], ident_f[:])
                    rowb = work.tile([P, P], F32, tag="rowb")
                    nc.vector.tensor_copy(rowb[:], pst[:])
                    if col0 is None:
                        nc.sync.dma_start(dst_dram[sl, :], rowb[:])
                    else:
                        nc.sync.dma_start(dst_dram[sl, col0:col0 + H], rowb[:])

            # ---- AllGather fat table ----
            nc.gpsimd.collective_compute(
                "AllGather", mybir.AluOpType.bypass, replica_groups=rg,
                ins=[fat_shard.opt()], outs=[fat_table],
            )

            # ---- edge loop ----
            msgT = big.tile([P, NSH], F32, tag="ovl_msgT")
            msg_rows = big.tile([P, NSH], F32, tag="ovl_rows")
            ICOLS = GT * P // 16  # idx16 columns per gather group
            for g in range(cfg.NG):
                half = (g * GT) // HT
                tab = fat_table[half * NHALF:(half + 1) * NHALF, :]
                isl = slice(g * ICOLS, (g + 1) * ICOLS)
                sidx = work.tile([P, ICOLS], mybir.dt.int16, tag="sidx")
                nc.sync.dma_start(sidx[:], io["src_idx16"][:, isl])
                didx = work.tile([P, ICOLS], mybir.dt.int16, tag="didx")
                nc.sync.dma_start(didx[:], io["dst_idx16"][:, isl])
                h_stage = stage.tile([P, GT * 2 * H], F32, tag="h_stage")
                hs3 = h_stage[:].rearrange("p (s e) -> p s e", e=2 * H)
                nc.gpsimd.dma_gather(
                    out_ap=hs3, in_ap=tab, idxs_ap=sidx[:],
                    num_idxs=GT * P, num_idxs_reg=GT * P, elem_size=2 * H)
                b_stage = stage.tile([P, GT * H], F32, tag="b_stage")
                bs3 = b_stage[:].rearrange("p (s e) -> p s e", e=H)
                nc.gpsimd.dma_gather(
                    out_ap=bs3, in_ap=b_local[:], idxs_ap=didx[:],
                    num_idxs=GT * P, num_idxs_reg=GT * P, elem_size=H)
                for bb in range(BPG):
                    b = g * BPG + bb
                    esl = slice(b * BLKT * P, (b + 1) * BLKT * P)
                    ein_sb = work.tile([7, BLKT * P], F32, tag="ein")
                    nc.sync.dma_start(ein_sb[:], io["einT"][:, esl])
                    pse = ps_big.tile([P, BLKT * P], F32, tag="psb")
                    nc.tensor.matmul(pse[:], lhsT=we_ext[:], rhs=ein_sb[:],
                                     start=True, stop=True)
                    eT_sb = work.tile([P, BLKT * P], BF, tag="eT")
                    nc.scalar.activation(eT_sb[:], pse[:], Silu)
                    psw = ps_big.tile([P, BLKT * P], F32, tag="psb")
                    for k in range(BLKT):
                        ks = slice(k * P, (k + 1) * P)
                        nc.tensor.matmul(psw[:, ks], lhsT=eT_sb[:, ks],
                                         rhs=wge_t[:, lp], start=True, stop=True)
                    # W_raw = E + B[dst] + A[src]; silu; m = h[src] * W
                    bsl3 = slice(bb * BLKT, (bb + 1) * BLKT)
                    t_sb = work.tile([P, BLKT * P], F32, tag="t_sb")
                    nc.vector.tensor_add(t_sb[:], psw[:],
                                         b_stage[:, bsl3.start * H:bsl3.stop * H])
                    w_sb = work.tile([P, BLKT * P], F32, tag="w_sb")
                    nc.vector.tensor_add(
                        w_sb[:].rearrange("p (s e) -> p s e", e=H), 
                        t_sb[:].rearrange("p (s e) -> p s e", e=H),
                        hs3[:, bsl3, H:2 * H])
                    ws2 = work.tile([P, BLKT * P], F32, tag="ws2")
                    nc.scalar.activation(ws2[:], w_sb[:], Silu)
                    m_sb = work.tile([P, BLKT * P], F32, tag="m_sb")
                    nc.vector.tensor_mul(
                        m_sb[:].rearrange("p (s e) -> p s e", e=H),
                        ws2[:].rearrange("p (s e) -> p s e", e=H),
                        hs3[:, bsl3, 0:H])
                    for k in range(BLKT):
                        j = b * BLKT + k
                        t, half_, gstart, gend = tile_meta(j)
                        s_sb = work.tile([P, P], F32, tag="s_sb")
                        nc.vector.tensor_tensor(
                            out=s_sb[:],
                            in0=dstcol[:, j:j + 1].to_broadcast([P, P]),
                            in1=iota_mat[:], op=mybir.AluOpType.is_equal)
                        if gstart:
                            pm_cur = ps_msg.tile([P, P], F32, tag="pm")
                        nc.tensor.matmul(pm_cur[:], lhsT=s_sb[:],
                                         rhs=m_sb[:, k * P:(k + 1) * P],
                                         start=gstart, stop=gend)
                        if gend:
                            msl = slice(t * P, (t + 1) * P)
                            if half_ == 0:
                                nc.scalar.copy(msg_rows[:, msl], pm_cur[:])
                            else:
                                nc.vector.tensor_add(msg_rows[:, msl],
                                                     msg_rows[:, msl], pm_cur[:])
            # transpose message rows -> msgT
            for t in range(NT):
                msl = slice(t * P, (t + 1) * P)
                pst = ps_tr.tile([P, P], F32, tag="pst")
                nc.tensor.transpose(pst[:], msg_rows[:, msl], ident_f[:])
                nc.vector.tensor_copy(msgT[:, msl], pst[:])

            # ---- GraphNorm stats + AllReduce ----
            st = small.tile([P, 2], F32, tag="st")
            nc.vector.tensor_reduce(st[:, 0:1], msgT[:], axis=mybir.AxisListType.X,
                                    op=mybir.AluOpType.add)
            nc.scalar.activation(msg_rows[:], msgT[:], Square, accum_out=st[:, 1:2])
            nc.sync.dma_start(stats_in[:], st[:])
            nc.gpsimd.collective_compute(
                "AllReduce", mybir.AluOpType.add, replica_groups=rg,
                ins=[stats_in.opt()], outs=[stats_out],
            )
            stg = small.tile([P, 2], F32, tag="stg")
            nc.sync.dma_start(stg[:], stats_out)

            # scalars: alpha_c, beta_c  [P,1]
            inv_n = 1.0 / cfg.N
            m1 = small.tile([P, 1], F32, tag="m1")
            nc.vector.tensor_scalar_mul(m1[:], stg[:, 0:1], inv_n)
            t1 = small.tile([P, 1], F32, tag="t1")
            nc.vector.tensor_mul(t1[:], gnms_c[:, l:l + 1], m1[:])
            e2 = small.tile([P, 1], F32, tag="e2")
            nc.vector.tensor_scalar_mul(e2[:], stg[:, 1:2], inv_n)
            q = small.tile([P, 1], F32, tag="q")
            nc.vector.tensor_mul(q[:], m1[:], t1[:])
            p2 = small.tile([P, 1], F32, tag="p2")
            nc.vector.tensor_mul(p2[:], t1[:], t1[:])
            var = small.tile([P, 1], F32, tag="var")
            nc.vector.tensor_scalar(out=var[:], in0=q[:], scalar1=-2.0, scalar2=None,
                                    op0=mybir.AluOpType.mult)
            nc.vector.tensor_add(var[:], var[:], e2[:])
            nc.vector.tensor_add(var[:], var[:], p2[:])
            nc.vector.tensor_scalar_add(var[:], var[:], EPS)
            sd = small.tile([P, 1], F32, tag="sd")
            nc.scalar.activation(sd[:], var[:], Sqrt)
            rinv = small.tile([P, 1], F32, tag="rinv")
            nc.vector.reciprocal(rinv[:], sd[:])
            alpha_c = small.tile([P, 1], F32, tag="alpha_c")
            nc.vector.tensor_mul(alpha_c[:], gnw_c[:, l:l + 1], rinv[:])
            bb_ = small.tile([P, 1], F32, tag="bb_")
            nc.vector.tensor_mul(bb_[:], t1[:], alpha_c[:])
            beta_c = small.tile([P, 1], F32, tag="beta_c")
            nc.vector.tensor_tensor(out=beta_c[:], in0=gnb_c[:, l:l + 1], in1=bb_[:],
                                    op=mybir.AluOpType.subtract)

            # ---- gn + swish + node MLPs ----
            x1 = big.tile([P, NSH], BF, tag="x1")
            for (o, c) in chunks:
                nc.scalar.activation(x1[:, o:o + c], msgT[:, o:o + c], Silu,
                                     bias=beta_c[:], scale=alpha_c[:])
            x2 = big.tile([P, NSH], BF, tag="x2")
            for (o, c) in chunks:
                ps1 = ps_big.tile([P, cfg.CHUNK], F32, tag="psb")
                nc.tensor.matmul(ps1[:, :c], lhsT=wlinh_t[:, lp], rhs=x1[:, o:o + c],
                                 start=True, stop=True)
                nc.scalar.activation(x2[:, o:o + c], ps1[:, :c], Silu,
                                     bias=blinh_c[:, l:l + 1])
            for (o, c) in chunks:
                ps2_ = ps_big.tile([P, cfg.CHUNK], F32, tag="psb")
                nc.tensor.matmul(ps2_[:, :c], lhsT=wmlp_t[:, lp], rhs=x2[:, o:o + c],
                                 start=True, stop=True)
                nc.scalar.activation(hT[:, o:o + c], ps2_[:, :c], Silu,
                                     bias=bmlp_c[:, l:l + 1])


# ====================== host side ======================

def _prep_weights(cfg: Cfg, w):
    L = cfg.L
    out = {}
    we = np.zeros((7, P), np.float32)
    we[0:3, 0:64] = w["w_e1"].T          # [64,3] -> [3,64]
    we[3:6, 64:128] = w["w_e12"].T       # [64,3] -> [3,64]
    we[6, 0:64] = w["b_e1"]
    we[6, 64:128] = w["b_e12"]
    out["we_ext"] = we
    wh = np.zeros((5, P), np.float32)
    wh[0:3, 0:64] = w["w_h1"].T
    wh[3:4, 64:128] = w["w_h12"].T
    wh[4, 0:64] = w["b_h1"]
    wh[4, 64:128] = w["b_h12"]
    out["wh_ext"] = wh

    def packT(mat):  # [L, out, in] -> [in, L*out] transposes along free axis
        return np.concatenate([mat[i].T for i in range(L)], axis=1).astype(BF16)

    out["wge_t"] = packT(w["w_geom"][:, :, 0:NF])
    out["wgs_t"] = packT(w["w_geom"][:, :, NF:NF + H]).astype(np.float32)
    out["wgd_t"] = packT(w["w_geom"][:, :, NF + H:NF + 2 * H]).astype(np.float32)
    out["wlinh_t"] = packT(w["w_linh"])
    out["wmlp_t"] = packT(w["w_mlp"])
    wo = np.zeros((P, 65), np.float32)
    wo[:, 0:64] = w["w_lin1"].T
    wo[:, 64] = w["w_wlin"][0]
    out["wout_pack"] = wo
    out["wlin2_t"] = w["w_lin2"].T.astype(BF16)
    out["bgeom_c"] = w["b_geom"].T.astype(np.float32).copy()    # [P, L]
    out["blinh_c"] = w["b_linh"].T.astype(np.float32).copy()
    out["bmlp_c"] = w["b_mlp"].T.astype(np.float32).copy()
    out["gnw_c"] = w["gn_w"].T.astype(np.float32).copy()
    out["gnb_c"] = w["gn_b"].T.astype(np.float32).copy()
    out["gnms_c"] = w["gn_ms"].T.astype(np.float32).copy()
    out["blin1_c"] = w["b_lin1"][:, None].astype(np.float32)
    out["bscal"] = np.array([[w["b_wlin"][0], w["b_lin2"][0]]], np.float32)
    return out


def preprocess(cfg: Cfg, inputs):
    """Full inputs -> list of per-core in_maps + host-side leftovers."""
    pos = np.asarray(inputs["pos"], np.float32)
    forces = np.asarray(inputs["forces"], np.float32)
    beam = np.asarray(inputs["beam_col"], np.float32)
    ei = np.asarray(inputs["edge_index"]).astype(np.int64)
    batch = np.asarray(inputs["batch"]).astype(np.int64)
    src, dst = ei[0], ei[1]
    E = src.shape[0]
    NSH = cfg.NSH

    rel = pos[src] - pos[dst]
    ew = np.sqrt((rel * rel).sum(1))
    ein_all = np.concatenate(
        [rel, beam, ew[:, None], np.ones((E, 1), np.float32)], axis=1
    ).astype(np.float32)  # [E, 7]

    fn = np.sqrt((forces * forces).sum(1))
    node_in = np.concatenate(
        [forces, fn[:, None], np.ones((len(fn), 1), np.float32)], axis=1)  # [N,5]
    node_in_pad = np.zeros((cfg.NPAD, 5), np.float32)
    node_in_pad[:cfg.N] = node_in

    wmap = _prep_weights(cfg, inputs)

    owner = np.minimum(dst // NSH, cfg.NCORES - 1)
    NHALF = cfg.NHALF
    HT, TPNH, NT = cfg.HT, cfg.TPNH, cfg.NT
    GTP = cfg.GATH_T * P
    in_maps = []
    for c in range(cfg.NCORES):
        sel = np.nonzero(owner == c)[0]
        ldst = dst[sel] - c * NSH
        t_of = ldst >> 7
        half_of = (src[sel] >= NHALF).astype(np.int64)
        einT = np.zeros((7, cfg.ES), np.float32)
        src_i = np.zeros((cfg.ES,), np.int64)
        dst_i = np.zeros((cfg.ES,), np.int64)
        dcol = np.full((cfg.ES,), -1.0, np.float32)
        for hh in range(2):
            for t in range(NT):
                gsel = sel[(t_of == t) & (half_of == hh)]
                cnt = len(gsel)
                cap = TPNH * P
                if cnt > cap:
                    raise RuntimeError(f"core {c} nt {t} half {hh}: {cnt} > {cap}")
                base = hh * HT * P + t * TPNH * P
                einT[:, base:base + cnt] = ein_all[gsel].T
                src_i[base:base + cnt] = src[gsel] - hh * NHALF
                dst_i[base:base + cnt] = dst[gsel] - c * NSH
                dcol[base:base + cnt] = (dst[gsel] - c * NSH) & 127
        # pack idx16: edge e -> [e%16, e//16], grouped per gather group
        def pack16(vals):
            out = np.zeros((16, cfg.ES // 16), np.uint16)
            e = np.arange(cfg.ES)
            out[e % 16, e // 16] = vals.astype(np.uint16)
            return np.tile(out, (8, 1)).view(np.int16)
        im = {
            "einT": einT,
            "node_inT": node_in_pad[c * NSH:(c + 1) * NSH].T.copy(),
            "src_idx16": pack16(src_i),
            "dst_idx16": pack16(dst_i),
            "dstcol": dcol.reshape(cfg.ET, P).T.copy(),
        }
        im.update(wmap)
        in_maps.append(im)
    return in_maps, batch


def postprocess(cfg: Cfg, inputs, results, batch):
    h = np.concatenate([r["out_h"] for r in results], axis=0)[:cfg.N]
    contrib = np.concatenate([r["out_contrib"] for r in results], axis=1)[:, :cfg.N]
    w_skip = np.asarray(inputs["w_skip"], np.float32)
    b_skip = np.asarray(inputs["b_skip"], np.float32)
    energies = np.zeros((cfg.G, cfg.L + 1), np.float32)
    for ll in range(cfg.L + 1):
        np.add.at(energies[:, ll], batch, contrib[ll])
    energy = energies @ w_skip.T + b_skip
    return energy.astype(np.float32), h.astype(np.float32)


_CACHE = {}


def build_nc(cfg: Cfg):
    nc = bacc.Bacc("TRN2", target_bir_lowering=False, debug=False)
    io = {}
    for name, (shp, dt) in input_specs(cfg).items():
        io[name] = nc.dram_tensor(
            name, shp, mybir.dt.from_np(np.dtype(dt)), kind="ExternalInput").ap()
    for name, (shp, dt) in output_specs(cfg).items():
        io[name] = nc.dram_tensor(
            name, shp, mybir.dt.from_np(np.dtype(dt)), kind="ExternalOutput").ap()
    with tile.TileContext(nc) as tc:
        emit(tc, io, cfg)
    nc.compile()
    return nc


def _build_cached_call(nc, n_cores):
    """Build the SPMD jitted callable once (mirrors bass2jax.run_bass_via_pjrt)
    so repeat kernel() calls skip re-tracing/compile-cache lookups."""
    import jax
    from jax.sharding import Mesh, PartitionSpec
    from jax.experimental.shard_map import shard_map
    from concourse import bass2jax, mybir as _mb

    bass2jax.install_neuronx_cc_hook()
    in_names, out_names, out_avals, zero_shapes = [], [], [], []
    partition_name = nc.partition_id_tensor.name if nc.partition_id_tensor else None
    for alloc in nc.m.functions[0].allocations:
        if not isinstance(alloc, _mb.MemoryLocationSet):
            continue
        name = alloc.memorylocations[0].name
        if alloc.kind == "ExternalInput":
            if name != partition_name:
                in_names.append(name)
        elif alloc.kind == "ExternalOutput":
            out_names.append(name)
            shape = tuple(alloc.tensor_shape)
            dtype = _mb.dt.np(alloc.dtype)
            out_avals.append(jax.core.ShapedArray(shape, dtype))
            zero_shapes.append((shape, dtype))
    n_params = len(in_names)
    all_names = list(in_names) + list(out_names)
    if partition_name is not None:
        all_names.append(partition_name)

    def _body(*args):
        operands = list(args)
        if partition_name is not None:
            operands.append(bass2jax.partition_id_tensor())
        outs = bass2jax._bass_exec_p.bind(
            *operands, out_avals=tuple(out_avals), in_names=tuple(all_names),
            out_names=tuple(out_names), lowering_input_output_aliases=(),
            sim_require_finite=True, sim_require_nnan=True, nc=nc)
        return tuple(outs)

    devices = jax.devices()[:n_cores]
    mesh = Mesh(np.asarray(devices), ("core",))
    n_outs = len(out_names)
    in_specs = (PartitionSpec("core"),) * (n_params + n_outs)
    out_specs = (PartitionSpec("core"),) * n_outs
    sharded = jax.jit(shard_map(_body, mesh=mesh, in_specs=in_specs,
                                out_specs=out_specs, check_rep=False),
                      keep_unused=True)

    def call(in_maps):
        concat_in = [np.concatenate([np.asarray(m[n]) for m in in_maps], axis=0)
                     for n in in_names]
        concat_zeros = [np.zeros((n_cores * s[0], *s[1:]), d)
                        for (s, d) in zero_shapes]
        out_arrs = sharded(*concat_in, *concat_zeros)
        return [{n: np.asarray(out_arrs[i]).reshape(n_cores, *zero_shapes[i][0])[c]
                 for i, n in enumerate(out_names)} for c in range(n_cores)]

    return call


def kernel(**inputs):
    cfg = FULL
    in_maps, batch = preprocess(cfg, inputs)
    if "nc" not in _CACHE:
        _CACHE["nc"] = build_nc(cfg)
    try:
        if "call" not in _CACHE:
            _CACHE["call"] = _build_cached_call(_CACHE["nc"], cfg.NCORES)
        results = _CACHE["call"](in_maps)
    except Exception:
        _CACHE.pop("call", None)
        res = run_bass_kernel_spmd(
            _CACHE["nc"], in_maps, core_ids=list(range(cfg.NCORES)))
        results = res.results
    return postprocess(cfg, inputs, results, batch)
